# revision 51
# baseline (speedup 1.0000x reference)
"""Trainium2 Bass kernel for the FNO-SMM problem (nn_FNO_SMM_34488587387600), v4.

Data-parallel over 8 NeuronCores: 2 batches per core. The V build and fc0
move to the host: vt (fp8, pair-chunk layout for DoubleRow), vinv (fp8,
m-major) and h0 (both layouts) are precomputed in numpy and DMA'd in
(batched, latency-ordered, halves pipelined).

Per core, per layer:
  - forward NUDFT: fp8 DoubleRow matmuls, batch-outer so b0 starts as soon
    as its vt half lands.
  - mode mix: 288 compact [64,64] augmented-complex matmuls.
  - extraction + packed coefficient slabs -> CT tiles.
  - transposed inverse NUDFT (stationary vinv fp8 chunks, moving CT f16)
    + 1x1 conv (bias via ones-row) -> piT PSUM -> Act gelu -> hT16; Pool
    casts hT16->hT8. The hT16->h transposes + copies for both batches are
    deferred until after the inverse matmuls so the PE never waits on a
    per-group gelu.
  - fc1/fc2 head, output DMA'd straight from PSUM.
"""
import sys
import os

sys.path.insert(0, '/opt/trn_rl_repo')

import numpy as np
import ml_dtypes
from contextlib import ExitStack

import concourse.bass as bass
import concourse.tile as tile
from concourse import bacc, mybir
from concourse.bass_utils import run_bass_kernel_spmd

MODES = 12
C = 32
N = 4096
B = 16
NCORES = 8
BL = B // NCORES          # 2 batches per core
NW = 299                  # working-set rows: 288 + 11 unpaired
NWP = 304                 # padded
NQ = 16                   # fwd pair-chunks (256 points each)
VTW = NQ * 1216           # vt cols per batch

F32 = mybir.dt.float32
F16 = mybir.dt.float16
F8 = mybir.dt.float8e4
AF = mybir.ActivationFunctionType
ALU = mybir.AluOpType
PM = mybir.MatmulPerfMode

F8NP = ml_dtypes.float8_e4m3fn

TRACE = False

_CACHE = {}


def _w_rows():
    return list(range(288)) + [24 * j + 12 for j in range(12, 23)]


def mode_col(u):
    a, s = divmod(u, 12)
    f = 23 * a + s
    if f < 288:
        return f, False
    i, j = f % 24, f // 24
    if i == 12:
        return 288 + (j - 12), False
    return 24 * (23 - j) + ((24 - i) % 24), True


def _cap(t_ap, row0, nrows, pairs, free_off):
    base = t_ap.ap
    pstep = base[0][0]
    return bass.AP(tensor=t_ap.tensor, offset=row0 * pstep + free_off + t_ap.offset,
                   ap=[[pstep, nrows]] + [list(p) for p in pairs])


def _build_program():
    nc = bacc.Bacc("TRN2", target_bir_lowering=False, debug=False,
                   num_devices=NCORES)

    din = {}
    def dram_in(name, shape, dt):
        din[name] = nc.dram_tensor(name, list(shape), dt, kind="ExternalInput").ap()
        return din[name]

    vt_d = dram_in('vt8', [BL, 128, VTW], F8)
    vi_d = dram_in('vi8', [BL, 128, 5 * N], F8)
    ht0_d = dram_in('ht0', [128, BL * 1024], F8)
    h0c_d = dram_in('h0c', [33, BL * N], F16)
    mmw_d = dram_in('mmw2', [4, 2, 64, 9216], F16)
    b16_d = dram_in('b16', [128, 385], F16)
    b32_d = dram_in('b32', [128, 65], F32)

    # y[b, n] lives at y_d[b, n % 128, n // 128] (p-major for fast DMA)
    y_d = nc.dram_tensor('y', [BL, 128, 32], F32, kind="ExternalOutput").ap()

    mcols = [mode_col(u)[0] for u in range(288)]

    with tile.TileContext(nc) as tc, ExitStack() as ctx:
        # ------------- persistent pool -------------
        pp = ctx.enter_context(tc.tile_pool(name="persist", bufs=1))
        vt = [pp.tile([128, VTW], F8, tag=f"vt{b}", name=f"vt{b}")
              for b in range(BL)]
        vinv = [pp.tile([128, 5 * N], F8, tag=f"vi{b}", name=f"vi{b}")
                for b in range(BL)]
        hT16 = pp.tile([128, BL * 1024], F16, tag="hT16", name="hT16")
        hT8 = pp.tile([128, BL * 1024], F8, tag="hT8", name="hT8")
        hh = pp.tile([33, BL * N], F16, tag="hh", name="hh")
        h = [hh[:, b * N:(b + 1) * N] for b in range(BL)]
        CT = [[pp.tile([128, C], F16, tag=f"CT{b}_{t}", name=f"CT{b}_{t}")
               for t in range(5)] for b in range(BL)]

        b16 = pp.tile([128, 385], F16, tag="b16", name="b16")
        b32 = pp.tile([128, 65], F32, tag="b32", name="b32")
        i128_t = b16[:, 0:128]
        cwtb_t = [b16[0:33, 128 + 32 * l:160 + 32 * l] for l in range(4)]
        fc1w_t = b16[0:C, 256:384]
        fc2w_t = b16[:, 384:385]
        is32_t = b32[0:C, 0:32]
        js32_t = b32[0:C, 32:64]
        fc1b_t = b32[:, 64:65]

        # ------------- DMA schedule (order = queue order) -------------
        nc.sync.dma_start(hT8[:], ht0_d[:])
        for k in range(4):
            nc.sync.dma_start(vt[0][:, VTW // 4 * k:VTW // 4 * (k + 1)],
                              vt_d[0, :, VTW // 4 * k:VTW // 4 * (k + 1)])
        nc.sync.dma_start(vt[1][:, 0:VTW // 2], vt_d[1, :, 0:VTW // 2])
        nc.sync.dma_start(vt[1][:, VTW // 2:], vt_d[1, :, VTW // 2:])

        with tc.tile_pool(name="work", bufs=1) as wk, \
             tc.tile_pool(name="wkps", bufs=1, space="PSUM") as wkps:

            def slab_dma(l):
                sE = wk.tile([64, 9216], F16, tag="sE", bufs=2, name=f"sE{l}")
                sO = wk.tile([64, 9216], F16, tag="sO", bufs=2, name=f"sO{l}")
                nc.sync.dma_start(sE[:], mmw_d[l, 0])
                nc.sync.dma_start(sO[:], mmw_d[l, 1])
                return sE, sO

            slabs_next = slab_dma(0)
            nc.sync.dma_start(b16[:], b16_d[:])
            nc.sync.dma_start(b32[:], b32_d[:])
            nc.sync.dma_start(hh[:], h0c_d[:])
            for b in range(BL):
                vsrc = vi_d[b].rearrange("p (t n) -> p t n", t=5)
                vdst = vinv[b][:].rearrange("p (t n) -> p t n", t=5)
                nc.sync.dma_start(vdst[:, :, 0:N // 2], vsrc[:, :, 0:N // 2])
                nc.sync.dma_start(vdst[:, :, N // 2:], vsrc[:, :, N // 2:])

            pending = []        # deferred transpose emitters from layer l-1
            for l in range(4):
                last = (l == 3)
                sE, sO = slabs_next
                if not last:
                    slabs_next = slab_dma(l + 1)

                # ---- forward NUDFT: fp8 DoubleRow, batch-outer ----
                big = wkps.tile([128, 2048], F32, tag="pxpm", name=f"pxpm{l}")
                for b in range(BL):
                    for q in range(NQ):
                        lhs = hT8[:, 1024 * b + 64 * q:1024 * b + 64 * (q + 1)] \
                            .rearrange("p (two f) -> p two f", two=2)
                        for half in range(2):
                            base = 1216 * q + 608 * half
                            rhs = vt[b][:, base:base + 608].rearrange(
                                "p (two f) -> p two f", two=2)
                            out = big[0:32, 1024 * b + 512 * half:
                                      1024 * b + 512 * half + NWP]
                            nc.tensor.matmul(out, lhs, rhs,
                                             start=(q == 0), stop=(q == NQ - 1),
                                             perf_mode=PM.DoubleRow)

                # ---- x_ft slab ----
                xs2 = wk.tile([64, 2 * NWP], F16, tag="xs2", name=f"xs2_{l}")
                for b in range(BL):
                    nc.vector.tensor_copy(
                        _cap(xs2, 0, 32, [[2, NWP]], b),
                        big[0:32, 1024 * b:1024 * b + NWP])
                    nc.scalar.activation(
                        _cap(xs2, 32, 32, [[2, NWP]], b),
                        big[0:32, 1024 * b + 512:1024 * b + 512 + NWP], AF.Copy)
                # fill the xs2/mix wait with last layer's deferred transposes
                for fn_ in pending[0:4]:
                    fn_()

                # ---- mode mix ----
                pm = big
                for c4 in range(4):
                    for rr in range(36):
                        r = 36 * c4 + rr
                        for par in range(2):
                            u = 2 * r + par
                            mc = mcols[u]
                            st = (sE if par == 0 else sO)
                            nc.tensor.matmul(pm[0:64, 2 * u:2 * u + 2],
                                             st[:, 2304 * c4 + 64 * rr:
                                                2304 * c4 + 64 * (rr + 1)],
                                             xs2[:, 2 * mc:2 * mc + 2],
                                             start=True, stop=True)
                    if c4 == 1 and len(pending) == 8:
                        pending[4](); pending[5]()
                    if c4 == 3 and len(pending) == 8:
                        pending[6](); pending[7]()
                pending = []

                # ---- per batch: extraction -> CT -> inverse; transposes
                #      deferred so PE never waits on a per-group gelu ----
                frs = [wk.tile([C, NWP], F32, tag=f"frs{b}", name=f"frs{l}_{b}")
                       for b in range(BL)]
                fis = [wk.tile([C, NWP], F32, tag=f"fis{b}", name=f"fis{l}_{b}")
                       for b in range(BL)]
                frx = [wk.tile([C, NWP], F32, tag=f"frx{b}", name=f"frx{l}_{b}")
                       for b in range(BL)]
                fix = [wk.tile([C, NWP], F32, tag=f"fix{b}", name=f"fix{l}_{b}")
                       for b in range(BL)]
                tspec = [[(0, 0, 128, 0)], [(0, 128, 128, 0)],
                         [(0, 256, 48, 0), (1, 0, 64, 64)],
                         [(1, 64, 128, 0)], [(1, 192, 112, 0)]]
                pht = wkps.tile([32, 2048], F16, tag="ph", name=f"ph{l}")
                piT2 = wkps.tile([128, 512], F32, tag="piT", name=f"piT{l}")
                ct_eng = 0
                cp_eng = 0
                pc_slot = 0
                ph_slot = 0
                piT_slot = 0

                def extraction(b):
                    nonlocal ct_eng, pc_slot
                    nc.gpsimd.memset(frs[b][:, 288:NWP], 0.0)
                    nc.gpsimd.memset(fis[b][:, 288:NWP], 0.0)
                    nc.gpsimd.memset(frx[b][:], 0.0)
                    nc.gpsimd.memset(fix[b][:], 0.0)
                    nc.vector.tensor_copy(frs[b][:, 0:288],
                                          _cap(pm, 0, 32, [[2, 288]], b))
                    nc.scalar.activation(fis[b][:, 0:288],
                                         _cap(pm, 32, 32, [[2, 288]], b),
                                         AF.Copy)
                    def _cpy(o, i, eng):
                        if eng == 'act':
                            nc.scalar.activation(o, i, AF.Copy)
                        elif eng == 'pool':
                            nc.gpsimd.tensor_copy(o, i)
                        else:
                            nc.vector.tensor_copy(o, i)
                    for (dst, src_, e1, e2) in (
                            (frx[b], frs[b], 'act', 'pool'),
                            (fix[b], fis[b], 'dve', 'pool')):
                        d3 = dst[:, 0:288].rearrange("p (j i) -> p j i", i=24)
                        s3 = src_[:, 0:288].rearrange("p (j i) -> p j i", i=24)
                        _cpy(d3[:, 1:12, 1:12], s3[:, 1:12, 0:11], e1)
                        _cpy(d3[:, 1:12, 13:24], s3[:, 1:12, 12:23], e2)
                        _cpy(d3[:, 1:12, 0:1], s3[:, 1:12, 23:24], e1)
                        _cpy(dst[:, 288:299],
                             s3[:, 11:0:-1, 11:12].rearrange("p j i -> p (j i)"),
                             e1)
                    nc.gpsimd.tensor_scalar(fix[b][:, 288:299],
                                            fix[b][:, 288:299],
                                            -1.0, None, op0=ALU.mult)
                    if l == 0:
                        nc.gpsimd.memset(CT[b][2][32:64, :], 0.0)
                        nc.gpsimd.memset(CT[b][4][96:128, :], 0.0)
                    for t in range(5):
                        for (kind, c0, wdt, r0) in tspec[t]:
                            sd = frs[b] if kind == 0 else fis[b]
                            sf = frx[b] if kind == 0 else fix[b]
                            pc = big[:, 1024 + 32 * pc_slot:1056 + 32 * pc_slot]
                            pc_slot = (pc_slot + 1) % 4
                            nc.tensor.matmul(pc[0:wdt, :], sd[:, c0:c0 + wdt],
                                             is32_t, start=True, stop=False,
                                             is_transpose=True)
                            nc.tensor.matmul(pc[0:wdt, :], sf[:, c0:c0 + wdt],
                                             js32_t, start=False, stop=True,
                                             is_transpose=True)
                            dstap = CT[b][t][r0:r0 + wdt, :]
                            if ct_eng == 1:
                                nc.scalar.activation(dstap, pc[0:wdt, :],
                                                     AF.Copy,
                                                     scale=1.0 / 2048.0)
                            else:
                                nc.vector.tensor_scalar(dstap, pc[0:wdt, :],
                                                        1.0 / 2048.0, None,
                                                        op0=ALU.mult)
                            ct_eng = (ct_eng + 1) % 2

                def inverse(b, pys=None):
                    nonlocal piT_slot
                    for g4 in range(8):
                        piT4 = piT2[:, 128 * piT_slot:128 * (piT_slot + 1)]
                        piT_slot = (piT_slot + 1) % 4
                        for j in range(4):
                            ch = 4 * g4 + j
                            for t in range(5):
                                nc.tensor.matmul(
                                    piT4[:, 32 * j:32 * (j + 1)],
                                    vinv[b][:, N * t + 128 * ch:
                                            N * t + 128 * (ch + 1)],
                                    CT[b][t][:],
                                    start=(t == 0), stop=False)
                            nc.tensor.matmul(
                                piT4[:, 32 * j:32 * (j + 1)],
                                h[b][:, 128 * ch:128 * (ch + 1)],
                                cwtb_t[l], start=False, stop=True)
                        if g4 % 2 == 0:
                            continue
                        # one act/cast per pair of groups (slots adjacent)
                        s0 = (piT_slot - 2) % 4
                        src2 = piT2[:, 128 * s0:128 * s0 + 256]
                        dst2 = hT16[:, 1024 * b + 128 * (g4 - 1):
                                    1024 * b + 128 * (g4 + 1)]
                        if last:
                            nc.vector.tensor_copy(dst2, src2)
                            if pys is not None:
                                transpose_group(b, g4 - 1)
                                transpose_group(b, g4)
                                head_chunk(b, g4 // 2, pys)
                        else:
                            nc.scalar.activation(dst2, src2, AF.Gelu)
                            nc.gpsimd.tensor_copy(
                                hT8[:, 1024 * b + 128 * (g4 - 1):
                                    1024 * b + 128 * (g4 + 1)], dst2)

                def head_chunk(b, c4, pys):
                    pg = big[:, 1024 * (c4 % 2):1024 * (c4 % 2) + 1024]
                    for hhh in range(2):
                        nc.tensor.matmul(pg[:, 512 * hhh:512 * (hhh + 1)],
                                         fc1w_t,
                                         h[b][0:32, 1024 * c4 + 512 * hhh:
                                              1024 * c4 + 512 * (hhh + 1)],
                                         start=True, stop=True)
                    g = wk.tile([128, 1024], F16, tag="g", bufs=2,
                                name=f"g{b}_{c4}")
                    nc.scalar.activation(g[:], pg[:], AF.Gelu, bias=fc1b_t)
                    for k in range(8):
                        nc.tensor.matmul(
                            pys[:, 32 * b + 8 * c4 + k:32 * b + 8 * c4 + k + 1],
                            g[:, 128 * k:128 * (k + 1)],
                            fc2w_t, start=True, stop=True)

                def transpose_group(b, g4):
                    nonlocal cp_eng, ph_slot
                    ph = pht[0:32, 512 * ph_slot:512 * (ph_slot + 1)]
                    ph_slot = (ph_slot + 1) % 4
                    for j in range(4):
                        ch = 4 * g4 + j
                        nc.tensor.matmul(
                            ph[:, 128 * j:128 * (j + 1)],
                            hT16[:, 1024 * b + 32 * ch:
                                 1024 * b + 32 * (ch + 1)],
                            i128_t, start=True, stop=True,
                            is_transpose=True)
                    dst = h[b][0:32, 512 * g4:512 * (g4 + 1)]
                    if last or cp_eng == 0:
                        nc.vector.tensor_copy(dst, ph[:])
                    else:
                        nc.scalar.activation(dst, ph[:], AF.Copy)
                    cp_eng = (cp_eng + 1) % 2

                def transposes(b, pys=None):
                    for g4 in range(8):
                        transpose_group(b, g4)
                        if pys is not None and g4 % 2 == 1:
                            head_chunk(b, g4 // 2, pys)

                extraction(0)
                if last:
                    pys = wkps.tile([128, 64], F32, tag="pys", name="pys")
                    inverse(0, pys)
                else:
                    inverse(0)
                extraction(1)
                if last:
                    ys0 = wk.tile([128, 32], F32, tag="ys0", name="ys0")
                    nc.vector.tensor_copy(ys0[:], pys[:, 0:32])
                    nc.sync.dma_start(y_d[0], ys0[:])
                    inverse(1, pys)
                    ys1 = wk.tile([128, 32], F32, tag="ys1", name="ys1")
                    nc.vector.tensor_copy(ys1[:], pys[:, 32:64])
                    nc.sync.dma_start(y_d[1], ys1[:])
                else:
                    transposes(0)
                    inverse(1)
                    # b1's hT16->h transposes are only needed by the NEXT
                    # layer's conv; defer them into its fwd/xs2/mix windows.
                    pending = [lambda g4=g4: transpose_group(1, g4)
                               for g4 in range(8)]

    nc.compile()
    return nc


# --------------------------------------------------------------------------
# host marshaling
# --------------------------------------------------------------------------
def _marshal(pos, fc0_w, fc0_b, sw1r, sw1i, sw2r, sw2i, cw, cb,
             fc1_w, fc1_b, fc2_w, fc2_b):
    xp = (pos[:, :, 0] - pos[:, :, 0].min()).astype(np.float64)
    yp = (pos[:, :, 1] - pos[:, :, 1].min()).astype(np.float64)
    sx = np.float64(np.float32(6.28) / np.float32(xp.max()))
    sy = np.float64(np.float32(6.28) / np.float32(yp.max()))
    kx = np.concatenate([np.arange(MODES), np.arange(-MODES, 0)]).astype(np.float64)
    ky = np.concatenate([np.arange(MODES), np.arange(-(MODES - 1), 0)]).astype(np.float64)

    def wrap(v):
        return v - 2 * np.pi * np.round(v / (2 * np.pi))

    axw = np.stack([wrap(kx[i] * sx * xp).astype(np.float32) for i in range(24)],
                   axis=1)
    ayw = np.stack([wrap(ky[j] * sy * yp).astype(np.float32) for j in range(23)],
                   axis=1)

    worder = _w_rows()
    iw = np.array([m % 24 for m in worder])
    jw = np.array([m // 24 for m in worder])
    ph = axw[:, iw, :].astype(np.float64) + ayw[:, jw, :]
    cosW = np.zeros((B, NWP, N), np.float32)
    sinW = np.zeros((B, NWP, N), np.float32)
    cosW[:, :NW] = np.cos(ph)
    sinW[:, :NW] = -np.sin(ph)

    cs = np.stack([cosW, sinW], axis=1)                     # [B, half, NWP, N]
    csb = cs.reshape(B, 2, NWP, NQ, 2, 128)
    vt8 = np.ascontiguousarray(
        csb.transpose(0, 5, 3, 1, 4, 2)
    ).reshape(B, 128, VTW).astype(F8NP)

    vpk = np.zeros((B, 640, N), np.float32)
    vpk[:, 0:NWP] = cosW
    vpk[:, 320:320 + NWP] = sinW
    vi8 = np.ascontiguousarray(
        vpk.reshape(B, 5, 128, N).transpose(0, 2, 1, 3)
    ).reshape(B, 128, 5 * N).astype(F8NP)

    xin = np.stack([xp, yp], axis=-1)
    h0 = (xin @ fc0_w.astype(np.float64) + fc0_b.astype(np.float64))
    ht0 = np.ascontiguousarray(
        h0.reshape(B, 32, 128, C).transpose(0, 2, 1, 3)
    ).reshape(B, 128, 1024).astype(F8NP)
    h0c = np.zeros((B, 33, N), np.float16)
    h0c[:, 0:C] = h0.transpose(0, 2, 1).astype(np.float16)
    h0c[:, 32] = 1.0

    mmw2 = np.zeros((4, 2, 64, 9216), np.float16)
    for l in range(4):
        w1 = sw1r[l].astype(np.float64) + 1j * sw1i[l].astype(np.float64)
        w2 = sw2r[l].astype(np.float64) + 1j * sw2i[l].astype(np.float64)
        for u in range(288):
            a, s = u // 12, u % 12
            wm = w1[:, :, a, s] if a < 12 else w2[:, :, a - 12, s]
            wr = wm.real.astype(np.float16)
            wi = wm.imag.astype(np.float16)
            _, cj = mode_col(u)
            r, par = u // 2, u % 2
            blk = mmw2[l, par, :, 64 * r:64 * (r + 1)]
            blk[0:32, 0:32] = wr
            blk[0:32, 32:64] = wi
            if cj:
                blk[32:64, 0:32] = wi
                blk[32:64, 32:64] = -wr
            else:
                blk[32:64, 0:32] = -wi
                blk[32:64, 32:64] = wr

    # packed small-weight blobs
    b16 = np.zeros((128, 385), np.float16)
    b16[:, 0:128] = np.eye(128, dtype=np.float16)
    for l in range(4):
        b16[0:C, 128 + 32 * l:160 + 32 * l] = cw[l].T.astype(np.float16)
        b16[32, 128 + 32 * l:160 + 32 * l] = cb[l].astype(np.float16)
    b16[0:C, 256:384] = fc1_w.astype(np.float16)
    b16[:, 384] = fc2_w.reshape(128).astype(np.float16)
    b32 = np.zeros((128, 65), np.float32)
    eye32 = np.eye(C, dtype=np.float32)
    b32[0:C, 0:32] = eye32
    b32[0:C, 32:64] = eye32[::-1]
    b32[:, 64] = fc1_b.astype(np.float32)

    shared = dict(mmw2=mmw2, b16=b16, b32=b32)
    per_b = dict(vt8=vt8, vi8=vi8, ht0=ht0, h0c=h0c)
    return per_b, shared


def kernel(**inputs):
    per_b, shared = _marshal(**{k: np.asarray(v) for k, v in inputs.items()})

    if 'nc' not in _CACHE:
        _CACHE['nc'] = _build_program()
    nc = _CACHE['nc']

    in_maps = []
    for core in range(NCORES):
        m = dict(shared)
        s = slice(BL * core, BL * (core + 1))
        m['vt8'] = per_b['vt8'][s]
        m['vi8'] = per_b['vi8'][s]
        # ht0: [128, BL*1024] with batch at col offset 1024b
        m['ht0'] = np.ascontiguousarray(
            per_b['ht0'][s].transpose(1, 0, 2).reshape(128, BL * 1024))
        # h0c: [33, BL*N] with batch at col offset N*b
        m['h0c'] = np.ascontiguousarray(
            per_b['h0c'][s].transpose(1, 0, 2).reshape(33, BL * N))
        in_maps.append(m)

    res = run_bass_kernel_spmd(nc, in_maps, list(range(NCORES)), trace=TRACE)
    _CACHE['last_results'] = res

    fc2_b = np.asarray(inputs['fc2_b']).astype(np.float32)
    out = np.zeros((B, N, 1), np.float32)
    for core in range(NCORES):
        yv = res.results[core]['y']          # [BL, 128, 32]; n = 128j + p
        out[BL * core:BL * (core + 1), :, 0] = \
            yv.transpose(0, 2, 1).reshape(BL, N)
    out += fc2_b.reshape(1, 1, 1)
    return out


# revision 52
# speedup vs baseline: 1.0444x; 1.0444x over previous
"""Trainium2 Bass kernel for the FNO-SMM problem (nn_FNO_SMM_34488587387600), v4.

Data-parallel over 8 NeuronCores: 2 batches per core. The V build and fc0
move to the host: vt (fp8, pair-chunk layout for DoubleRow), vinv (fp8,
m-major) and h0 (both layouts) are precomputed in numpy and DMA'd in
(batched, latency-ordered, halves pipelined).

Per core, per layer:
  - forward NUDFT: fp8 DoubleRow matmuls, batch-outer so b0 starts as soon
    as its vt half lands.
  - mode mix: 288 compact [64,64] augmented-complex matmuls.
  - extraction + packed coefficient slabs -> CT tiles.
  - transposed inverse NUDFT (stationary vinv fp8 chunks, moving CT f16)
    + 1x1 conv (bias via ones-row) -> piT PSUM -> Act gelu -> hT16; Pool
    casts hT16->hT8. The hT16->h transposes + copies for both batches are
    deferred until after the inverse matmuls so the PE never waits on a
    per-group gelu.
  - fc1/fc2 head, output DMA'd straight from PSUM.
"""
import sys
import os

sys.path.insert(0, '/opt/trn_rl_repo')

import numpy as np
import ml_dtypes
from contextlib import ExitStack

import concourse.bass as bass
import concourse.tile as tile
from concourse import bacc, mybir
from concourse.bass_utils import run_bass_kernel_spmd

MODES = 12
C = 32
N = 4096
B = 16
NCORES = 8
BL = B // NCORES          # 2 batches per core
NW = 299                  # working-set rows: 288 + 11 unpaired
NWP = 304                 # padded
NQ = 16                   # fwd pair-chunks (256 points each)
VTW = NQ * 1216           # vt cols per batch

F32 = mybir.dt.float32
F16 = mybir.dt.float16
F8 = mybir.dt.float8e4
AF = mybir.ActivationFunctionType
ALU = mybir.AluOpType
PM = mybir.MatmulPerfMode

F8NP = ml_dtypes.float8_e4m3fn

TRACE = False

_CACHE = {}


def _w_rows():
    return list(range(288)) + [24 * j + 12 for j in range(12, 23)]


def mode_col(u):
    a, s = divmod(u, 12)
    f = 23 * a + s
    if f < 288:
        return f, False
    i, j = f % 24, f // 24
    if i == 12:
        return 288 + (j - 12), False
    return 24 * (23 - j) + ((24 - i) % 24), True


def _cap(t_ap, row0, nrows, pairs, free_off):
    base = t_ap.ap
    pstep = base[0][0]
    return bass.AP(tensor=t_ap.tensor, offset=row0 * pstep + free_off + t_ap.offset,
                   ap=[[pstep, nrows]] + [list(p) for p in pairs])


def _build_program():
    nc = bacc.Bacc("TRN2", target_bir_lowering=False, debug=False,
                   num_devices=NCORES)

    din = {}
    def dram_in(name, shape, dt):
        din[name] = nc.dram_tensor(name, list(shape), dt, kind="ExternalInput").ap()
        return din[name]

    vt_d = dram_in('vt8', [BL, 128, VTW], F8)
    vi_d = dram_in('vi8', [BL, 128, 5 * N], F8)
    ht0_d = dram_in('ht0', [128, BL * 1024], F8)
    h0c_d = dram_in('h0c', [33, BL * N], F16)
    mmw_d = dram_in('mmw2', [4, 2, 64, 9216], F16)
    b16_d = dram_in('b16', [128, 385], F16)
    b32_d = dram_in('b32', [128, 65], F32)

    # y[b, n] lives at y_d[b, n % 128, n // 128] (p-major for fast DMA)
    y_d = nc.dram_tensor('y', [BL, 128, 32], F32, kind="ExternalOutput").ap()

    mcols = [mode_col(u)[0] for u in range(288)]

    with tile.TileContext(nc) as tc, ExitStack() as ctx:
        # ------------- persistent pool -------------
        pp = ctx.enter_context(tc.tile_pool(name="persist", bufs=1))
        vt = [pp.tile([128, VTW], F8, tag=f"vt{b}", name=f"vt{b}")
              for b in range(BL)]
        vinv = [pp.tile([128, 5 * N], F8, tag=f"vi{b}", name=f"vi{b}")
                for b in range(BL)]
        hT16 = pp.tile([128, BL * 1024], F16, tag="hT16", name="hT16")
        hT8 = pp.tile([128, BL * 1024], F8, tag="hT8", name="hT8")
        hh = pp.tile([33, BL * N], F16, tag="hh", name="hh")
        h = [hh[:, b * N:(b + 1) * N] for b in range(BL)]
        CT = [[pp.tile([128, C], F16, tag=f"CT{b}_{t}", name=f"CT{b}_{t}")
               for t in range(5)] for b in range(BL)]

        b16 = pp.tile([128, 385], F16, tag="b16", name="b16")
        b32 = pp.tile([128, 65], F32, tag="b32", name="b32")
        i128_t = b16[:, 0:128]
        cwtb_t = [b16[0:33, 128 + 32 * l:160 + 32 * l] for l in range(4)]
        fc1w_t = b16[0:C, 256:384]
        fc2w_t = b16[:, 384:385]
        is32_t = b32[0:C, 0:32]
        js32_t = b32[0:C, 32:64]
        fc1b_t = b32[:, 64:65]

        # ------------- DMA schedule (order = queue order) -------------
        nc.sync.dma_start(hT8[:], ht0_d[:])
        for k in range(4):
            nc.sync.dma_start(vt[0][:, VTW // 4 * k:VTW // 4 * (k + 1)],
                              vt_d[0, :, VTW // 4 * k:VTW // 4 * (k + 1)])
        nc.sync.dma_start(vt[1][:, 0:VTW // 2], vt_d[1, :, 0:VTW // 2])
        nc.sync.dma_start(vt[1][:, VTW // 2:], vt_d[1, :, VTW // 2:])

        with tc.tile_pool(name="work", bufs=1) as wk, \
             tc.tile_pool(name="wkps", bufs=1, space="PSUM") as wkps:

            def slab_dma(l):
                sE = wk.tile([64, 9216], F16, tag="sE", bufs=2, name=f"sE{l}")
                sO = wk.tile([64, 9216], F16, tag="sO", bufs=2, name=f"sO{l}")
                nc.sync.dma_start(sE[:], mmw_d[l, 0])
                nc.sync.dma_start(sO[:], mmw_d[l, 1])
                return sE, sO

            slabs_next = slab_dma(0)
            nc.sync.dma_start(b16[:], b16_d[:])
            nc.sync.dma_start(b32[:], b32_d[:])
            nc.sync.dma_start(hh[:], h0c_d[:])
            for b in range(BL):
                vsrc = vi_d[b].rearrange("p (t n) -> p t n", t=5)
                vdst = vinv[b][:].rearrange("p (t n) -> p t n", t=5)
                nc.sync.dma_start(vdst[:, :, 0:N // 2], vsrc[:, :, 0:N // 2])
                nc.sync.dma_start(vdst[:, :, N // 2:], vsrc[:, :, N // 2:])

            pending = []        # deferred transpose emitters from layer l-1
            for l in range(4):
                last = (l == 3)
                sE, sO = slabs_next
                if not last:
                    slabs_next = slab_dma(l + 1)

                # ---- forward NUDFT: fp8 DoubleRow, batch-outer ----
                big = wkps.tile([128, 2048], F32, tag="pxpm", name=f"pxpm{l}")
                for b in range(BL):
                    for q in range(NQ):
                        lhs = hT8[:, 1024 * b + 64 * q:1024 * b + 64 * (q + 1)] \
                            .rearrange("p (two f) -> p two f", two=2)
                        for half in range(2):
                            base = 1216 * q + 608 * half
                            rhs = vt[b][:, base:base + 608].rearrange(
                                "p (two f) -> p two f", two=2)
                            out = big[0:32, 1024 * b + 512 * half:
                                      1024 * b + 512 * half + NWP]
                            nc.tensor.matmul(out, lhs, rhs,
                                             start=(q == 0), stop=(q == NQ - 1),
                                             perf_mode=PM.DoubleRow)

                # ---- x_ft slab ----
                xs2 = wk.tile([64, 2 * NWP], F16, tag="xs2", name=f"xs2_{l}")
                for b in range(BL):
                    nc.vector.tensor_copy(
                        _cap(xs2, 0, 32, [[2, NWP]], b),
                        big[0:32, 1024 * b:1024 * b + NWP])
                    nc.scalar.activation(
                        _cap(xs2, 32, 32, [[2, NWP]], b),
                        big[0:32, 1024 * b + 512:1024 * b + 512 + NWP], AF.Copy)
                # fill the xs2/mix wait with last layer's deferred transposes
                for fn_ in pending[0:4]:
                    fn_()

                # ---- mode mix ----
                pm = big
                for c4 in range(4):
                    for rr in range(36):
                        r = 36 * c4 + rr
                        for par in range(2):
                            u = 2 * r + par
                            mc = mcols[u]
                            st = (sE if par == 0 else sO)
                            nc.tensor.matmul(pm[0:64, 2 * u:2 * u + 2],
                                             st[:, 2304 * c4 + 64 * rr:
                                                2304 * c4 + 64 * (rr + 1)],
                                             xs2[:, 2 * mc:2 * mc + 2],
                                             start=True, stop=True)
                    if c4 == 1 and len(pending) == 8:
                        pending[4](); pending[5]()
                    if c4 == 3 and len(pending) == 8:
                        pending[6](); pending[7]()
                pending = []

                # ---- per batch: extraction -> CT -> inverse; transposes
                #      deferred so PE never waits on a per-group gelu ----
                frs = [wk.tile([C, NWP], F32, tag=f"frs{b}", name=f"frs{l}_{b}")
                       for b in range(BL)]
                fis = [wk.tile([C, NWP], F32, tag=f"fis{b}", name=f"fis{l}_{b}")
                       for b in range(BL)]
                frx = [wk.tile([C, NWP], F32, tag=f"frx{b}", name=f"frx{l}_{b}")
                       for b in range(BL)]
                fix = [wk.tile([C, NWP], F32, tag=f"fix{b}", name=f"fix{l}_{b}")
                       for b in range(BL)]
                tspec = [[(0, 0, 128, 0)], [(0, 128, 128, 0)],
                         [(0, 256, 48, 0), (1, 0, 64, 64)],
                         [(1, 64, 128, 0)], [(1, 192, 112, 0)]]
                pht = wkps.tile([32, 2048], F16, tag="ph", name=f"ph{l}")
                piT2 = wkps.tile([128, 512], F32, tag="piT", name=f"piT{l}")
                ct_eng = 0
                cp_eng = 0
                pc_slot = 0
                ph_slot = 0
                piT_slot = 0

                def extraction(b):
                    nonlocal ct_eng, pc_slot
                    nc.gpsimd.memset(frs[b][:, 288:NWP], 0.0)
                    nc.gpsimd.memset(fis[b][:, 288:NWP], 0.0)
                    nc.gpsimd.memset(frx[b][:], 0.0)
                    nc.gpsimd.memset(fix[b][:], 0.0)
                    nc.vector.tensor_copy(frs[b][:, 0:288],
                                          _cap(pm, 0, 32, [[2, 288]], b))
                    nc.scalar.activation(fis[b][:, 0:288],
                                         _cap(pm, 32, 32, [[2, 288]], b),
                                         AF.Copy)
                    def _cpy(o, i, eng):
                        if eng == 'act':
                            nc.scalar.activation(o, i, AF.Copy)
                        elif eng == 'pool':
                            nc.gpsimd.tensor_copy(o, i)
                        else:
                            nc.vector.tensor_copy(o, i)
                    for (dst, src_, e1, e2) in (
                            (frx[b], frs[b], 'act', 'pool'),
                            (fix[b], fis[b], 'dve', 'pool')):
                        d3 = dst[:, 0:288].rearrange("p (j i) -> p j i", i=24)
                        s3 = src_[:, 0:288].rearrange("p (j i) -> p j i", i=24)
                        _cpy(d3[:, 1:12, 1:12], s3[:, 1:12, 0:11], e1)
                        _cpy(d3[:, 1:12, 13:24], s3[:, 1:12, 12:23], e2)
                        _cpy(d3[:, 1:12, 0:1], s3[:, 1:12, 23:24], e1)
                        _cpy(dst[:, 288:299],
                             s3[:, 11:0:-1, 11:12].rearrange("p j i -> p (j i)"),
                             e1)
                    nc.gpsimd.tensor_scalar(fix[b][:, 288:299],
                                            fix[b][:, 288:299],
                                            -1.0, None, op0=ALU.mult)
                    if l == 0:
                        nc.gpsimd.memset(CT[b][2][32:64, :], 0.0)
                        nc.gpsimd.memset(CT[b][4][96:128, :], 0.0)
                    for t in range(5):
                        for (kind, c0, wdt, r0) in tspec[t]:
                            sd = frs[b] if kind == 0 else fis[b]
                            sf = frx[b] if kind == 0 else fix[b]
                            pc = big[:, 1024 + 32 * pc_slot:1056 + 32 * pc_slot]
                            pc_slot = (pc_slot + 1) % 4
                            nc.tensor.matmul(pc[0:wdt, :], sd[:, c0:c0 + wdt],
                                             is32_t, start=True, stop=False,
                                             is_transpose=True)
                            nc.tensor.matmul(pc[0:wdt, :], sf[:, c0:c0 + wdt],
                                             js32_t, start=False, stop=True,
                                             is_transpose=True)
                            dstap = CT[b][t][r0:r0 + wdt, :]
                            if ct_eng == 1:
                                nc.scalar.activation(dstap, pc[0:wdt, :],
                                                     AF.Copy,
                                                     scale=1.0 / 2048.0)
                            else:
                                nc.vector.tensor_scalar(dstap, pc[0:wdt, :],
                                                        1.0 / 2048.0, None,
                                                        op0=ALU.mult)
                            ct_eng = (ct_eng + 1) % 2

                def inverse(b):
                    nonlocal piT_slot
                    for g4 in range(8):
                        piT4 = piT2[:, 128 * piT_slot:128 * (piT_slot + 1)]
                        piT_slot = (piT_slot + 1) % 4
                        for j in range(4):
                            ch = 4 * g4 + j
                            for t in range(5):
                                nc.tensor.matmul(
                                    piT4[:, 32 * j:32 * (j + 1)],
                                    vinv[b][:, N * t + 128 * ch:
                                            N * t + 128 * (ch + 1)],
                                    CT[b][t][:],
                                    start=(t == 0), stop=False)
                            nc.tensor.matmul(
                                piT4[:, 32 * j:32 * (j + 1)],
                                h[b][:, 128 * ch:128 * (ch + 1)],
                                cwtb_t[l], start=False, stop=True)
                        if g4 % 2 == 0:
                            continue
                        # one act/cast per pair of groups (slots adjacent)
                        s0 = (piT_slot - 2) % 4
                        src2 = piT2[:, 128 * s0:128 * s0 + 256]
                        dst2 = hT16[:, 1024 * b + 128 * (g4 - 1):
                                    1024 * b + 128 * (g4 + 1)]
                        if last:
                            nc.vector.tensor_copy(dst2, src2)
                        else:
                            nc.scalar.activation(dst2, src2, AF.Gelu)
                            nc.gpsimd.tensor_copy(
                                hT8[:, 1024 * b + 128 * (g4 - 1):
                                    1024 * b + 128 * (g4 + 1)], dst2)

                def head_chunk(b, c4, pys):
                    pg = big[:, 1024 * (c4 % 2):1024 * (c4 % 2) + 1024]
                    for hhh in range(2):
                        nc.tensor.matmul(pg[:, 512 * hhh:512 * (hhh + 1)],
                                         fc1w_t,
                                         h[b][0:32, 1024 * c4 + 512 * hhh:
                                              1024 * c4 + 512 * (hhh + 1)],
                                         start=True, stop=True)
                    g = wk.tile([128, 1024], F16, tag="g", bufs=2,
                                name=f"g{b}_{c4}")
                    nc.scalar.activation(g[:], pg[:], AF.Gelu, bias=fc1b_t)
                    for k in range(8):
                        nc.tensor.matmul(
                            pys[:, 32 * b + 8 * c4 + k:32 * b + 8 * c4 + k + 1],
                            g[:, 128 * k:128 * (k + 1)],
                            fc2w_t, start=True, stop=True)

                def transpose_group(b, g4):
                    nonlocal cp_eng, ph_slot
                    ph = pht[0:32, 512 * ph_slot:512 * (ph_slot + 1)]
                    ph_slot = (ph_slot + 1) % 4
                    for j in range(4):
                        ch = 4 * g4 + j
                        nc.tensor.matmul(
                            ph[:, 128 * j:128 * (j + 1)],
                            hT16[:, 1024 * b + 32 * ch:
                                 1024 * b + 32 * (ch + 1)],
                            i128_t, start=True, stop=True,
                            is_transpose=True)
                    dst = h[b][0:32, 512 * g4:512 * (g4 + 1)]
                    if last or cp_eng == 0:
                        nc.vector.tensor_copy(dst, ph[:])
                    else:
                        nc.scalar.activation(dst, ph[:], AF.Copy)
                    cp_eng = (cp_eng + 1) % 2

                def transposes(b, pys=None):
                    for g4 in range(8):
                        transpose_group(b, g4)
                        if pys is not None and g4 % 2 == 1:
                            head_chunk(b, g4 // 2, pys)

                extraction(0)
                inverse(0)
                extraction(1)
                if last:
                    pys = wkps.tile([128, 64], F32, tag="pys", name="pys")
                    transposes(0, pys)
                    ys0 = wk.tile([128, 32], F32, tag="ys0", name="ys0")
                    nc.vector.tensor_copy(ys0[:], pys[:, 0:32])
                    nc.sync.dma_start(y_d[0], ys0[:])
                    inverse(1)
                    transposes(1, pys)
                    ys1 = wk.tile([128, 32], F32, tag="ys1", name="ys1")
                    nc.vector.tensor_copy(ys1[:], pys[:, 32:64])
                    nc.sync.dma_start(y_d[1], ys1[:])
                else:
                    transposes(0)
                    inverse(1)
                    # b1's hT16->h transposes are only needed by the NEXT
                    # layer's conv; defer them into its fwd/xs2/mix windows.
                    pending = [lambda g4=g4: transpose_group(1, g4)
                               for g4 in range(8)]

    nc.compile()
    return nc


# --------------------------------------------------------------------------
# host marshaling
# --------------------------------------------------------------------------
def _marshal(pos, fc0_w, fc0_b, sw1r, sw1i, sw2r, sw2i, cw, cb,
             fc1_w, fc1_b, fc2_w, fc2_b):
    xp = (pos[:, :, 0] - pos[:, :, 0].min()).astype(np.float64)
    yp = (pos[:, :, 1] - pos[:, :, 1].min()).astype(np.float64)
    sx = np.float64(np.float32(6.28) / np.float32(xp.max()))
    sy = np.float64(np.float32(6.28) / np.float32(yp.max()))
    kx = np.concatenate([np.arange(MODES), np.arange(-MODES, 0)]).astype(np.float64)
    ky = np.concatenate([np.arange(MODES), np.arange(-(MODES - 1), 0)]).astype(np.float64)

    def wrap(v):
        return v - 2 * np.pi * np.round(v / (2 * np.pi))

    axw = np.stack([wrap(kx[i] * sx * xp).astype(np.float32) for i in range(24)],
                   axis=1)
    ayw = np.stack([wrap(ky[j] * sy * yp).astype(np.float32) for j in range(23)],
                   axis=1)

    worder = _w_rows()
    iw = np.array([m % 24 for m in worder])
    jw = np.array([m // 24 for m in worder])
    ph = axw[:, iw, :].astype(np.float64) + ayw[:, jw, :]
    cosW = np.zeros((B, NWP, N), np.float32)
    sinW = np.zeros((B, NWP, N), np.float32)
    cosW[:, :NW] = np.cos(ph)
    sinW[:, :NW] = -np.sin(ph)

    cs = np.stack([cosW, sinW], axis=1)                     # [B, half, NWP, N]
    csb = cs.reshape(B, 2, NWP, NQ, 2, 128)
    vt8 = np.ascontiguousarray(
        csb.transpose(0, 5, 3, 1, 4, 2)
    ).reshape(B, 128, VTW).astype(F8NP)

    vpk = np.zeros((B, 640, N), np.float32)
    vpk[:, 0:NWP] = cosW
    vpk[:, 320:320 + NWP] = sinW
    vi8 = np.ascontiguousarray(
        vpk.reshape(B, 5, 128, N).transpose(0, 2, 1, 3)
    ).reshape(B, 128, 5 * N).astype(F8NP)

    xin = np.stack([xp, yp], axis=-1)
    h0 = (xin @ fc0_w.astype(np.float64) + fc0_b.astype(np.float64))
    ht0 = np.ascontiguousarray(
        h0.reshape(B, 32, 128, C).transpose(0, 2, 1, 3)
    ).reshape(B, 128, 1024).astype(F8NP)
    h0c = np.zeros((B, 33, N), np.float16)
    h0c[:, 0:C] = h0.transpose(0, 2, 1).astype(np.float16)
    h0c[:, 32] = 1.0

    mmw2 = np.zeros((4, 2, 64, 9216), np.float16)
    for l in range(4):
        w1 = sw1r[l].astype(np.float64) + 1j * sw1i[l].astype(np.float64)
        w2 = sw2r[l].astype(np.float64) + 1j * sw2i[l].astype(np.float64)
        for u in range(288):
            a, s = u // 12, u % 12
            wm = w1[:, :, a, s] if a < 12 else w2[:, :, a - 12, s]
            wr = wm.real.astype(np.float16)
            wi = wm.imag.astype(np.float16)
            _, cj = mode_col(u)
            r, par = u // 2, u % 2
            blk = mmw2[l, par, :, 64 * r:64 * (r + 1)]
            blk[0:32, 0:32] = wr
            blk[0:32, 32:64] = wi
            if cj:
                blk[32:64, 0:32] = wi
                blk[32:64, 32:64] = -wr
            else:
                blk[32:64, 0:32] = -wi
                blk[32:64, 32:64] = wr

    # packed small-weight blobs
    b16 = np.zeros((128, 385), np.float16)
    b16[:, 0:128] = np.eye(128, dtype=np.float16)
    for l in range(4):
        b16[0:C, 128 + 32 * l:160 + 32 * l] = cw[l].T.astype(np.float16)
        b16[32, 128 + 32 * l:160 + 32 * l] = cb[l].astype(np.float16)
    b16[0:C, 256:384] = fc1_w.astype(np.float16)
    b16[:, 384] = fc2_w.reshape(128).astype(np.float16)
    b32 = np.zeros((128, 65), np.float32)
    eye32 = np.eye(C, dtype=np.float32)
    b32[0:C, 0:32] = eye32
    b32[0:C, 32:64] = eye32[::-1]
    b32[:, 64] = fc1_b.astype(np.float32)

    shared = dict(mmw2=mmw2, b16=b16, b32=b32)
    per_b = dict(vt8=vt8, vi8=vi8, ht0=ht0, h0c=h0c)
    return per_b, shared


def kernel(**inputs):
    per_b, shared = _marshal(**{k: np.asarray(v) for k, v in inputs.items()})

    if 'nc' not in _CACHE:
        _CACHE['nc'] = _build_program()
    nc = _CACHE['nc']

    in_maps = []
    for core in range(NCORES):
        m = dict(shared)
        s = slice(BL * core, BL * (core + 1))
        m['vt8'] = per_b['vt8'][s]
        m['vi8'] = per_b['vi8'][s]
        # ht0: [128, BL*1024] with batch at col offset 1024b
        m['ht0'] = np.ascontiguousarray(
            per_b['ht0'][s].transpose(1, 0, 2).reshape(128, BL * 1024))
        # h0c: [33, BL*N] with batch at col offset N*b
        m['h0c'] = np.ascontiguousarray(
            per_b['h0c'][s].transpose(1, 0, 2).reshape(33, BL * N))
        in_maps.append(m)

    res = run_bass_kernel_spmd(nc, in_maps, list(range(NCORES)), trace=TRACE)
    _CACHE['last_results'] = res

    fc2_b = np.asarray(inputs['fc2_b']).astype(np.float32)
    out = np.zeros((B, N, 1), np.float32)
    for core in range(NCORES):
        yv = res.results[core]['y']          # [BL, 128, 32]; n = 128j + p
        out[BL * core:BL * (core + 1), :, 0] = \
            yv.transpose(0, 2, 1).reshape(BL, N)
    out += fc2_b.reshape(1, 1, 1)
    return out


# revision 53
# speedup vs baseline: 1.0466x; 1.0022x over previous
"""Trainium2 Bass kernel for the FNO-SMM problem (nn_FNO_SMM_34488587387600), v4.

Data-parallel over 8 NeuronCores: 2 batches per core. The V build and fc0
move to the host: vt (fp8, pair-chunk layout for DoubleRow), vinv (fp8,
m-major) and h0 (both layouts) are precomputed in numpy and DMA'd in
(batched, latency-ordered, halves pipelined).

Per core, per layer:
  - forward NUDFT: fp8 DoubleRow matmuls, batch-outer so b0 starts as soon
    as its vt half lands.
  - mode mix: 288 compact [64,64] augmented-complex matmuls.
  - extraction + packed coefficient slabs -> CT tiles.
  - transposed inverse NUDFT (stationary vinv fp8 chunks, moving CT f16)
    + 1x1 conv (bias via ones-row) -> piT PSUM -> Act gelu -> hT16; Pool
    casts hT16->hT8. The hT16->h transposes + copies for both batches are
    deferred until after the inverse matmuls so the PE never waits on a
    per-group gelu.
  - fc1/fc2 head, output DMA'd straight from PSUM.
"""
import sys
import os

sys.path.insert(0, '/opt/trn_rl_repo')

import numpy as np
import ml_dtypes
from contextlib import ExitStack

import concourse.bass as bass
import concourse.tile as tile
from concourse import bacc, mybir
from concourse.bass_utils import run_bass_kernel_spmd

MODES = 12
C = 32
N = 4096
B = 16
NCORES = 8
BL = B // NCORES          # 2 batches per core
NW = 299                  # working-set rows: 288 + 11 unpaired
NWP = 304                 # padded
NQ = 16                   # fwd pair-chunks (256 points each)
VTW = NQ * 1216           # vt cols per batch

F32 = mybir.dt.float32
F16 = mybir.dt.float16
F8 = mybir.dt.float8e4
AF = mybir.ActivationFunctionType
ALU = mybir.AluOpType
PM = mybir.MatmulPerfMode

F8NP = ml_dtypes.float8_e4m3fn

TRACE = False

_CACHE = {}


def _w_rows():
    return list(range(288)) + [24 * j + 12 for j in range(12, 23)]


def mode_col(u):
    a, s = divmod(u, 12)
    f = 23 * a + s
    if f < 288:
        return f, False
    i, j = f % 24, f // 24
    if i == 12:
        return 288 + (j - 12), False
    return 24 * (23 - j) + ((24 - i) % 24), True


def _cap(t_ap, row0, nrows, pairs, free_off):
    base = t_ap.ap
    pstep = base[0][0]
    return bass.AP(tensor=t_ap.tensor, offset=row0 * pstep + free_off + t_ap.offset,
                   ap=[[pstep, nrows]] + [list(p) for p in pairs])


def _build_program():
    nc = bacc.Bacc("TRN2", target_bir_lowering=False, debug=False,
                   num_devices=NCORES)

    din = {}
    def dram_in(name, shape, dt):
        din[name] = nc.dram_tensor(name, list(shape), dt, kind="ExternalInput").ap()
        return din[name]

    vt_d = dram_in('vt8', [BL, 128, VTW], F8)
    vi_d = dram_in('vi8', [BL, 128, 5 * N], F8)
    ht0_d = dram_in('ht0', [128, BL * 1024], F8)
    h0c_d = dram_in('h0c', [33, BL * N], F16)
    mmw_d = dram_in('mmw2', [4, 2, 64, 9216], F16)
    b16_d = dram_in('b16', [128, 385], F16)
    b32_d = dram_in('b32', [128, 65], F32)

    # y[b, n] lives at y_d[b, n % 128, n // 128] (p-major for fast DMA)
    y_d = nc.dram_tensor('y', [BL, 128, 32], F32, kind="ExternalOutput").ap()

    mcols = [mode_col(u)[0] for u in range(288)]

    with tile.TileContext(nc) as tc, ExitStack() as ctx:
        # ------------- persistent pool -------------
        pp = ctx.enter_context(tc.tile_pool(name="persist", bufs=1))
        vt = [pp.tile([128, VTW], F8, tag=f"vt{b}", name=f"vt{b}")
              for b in range(BL)]
        vinv = [pp.tile([128, 5 * N], F8, tag=f"vi{b}", name=f"vi{b}")
                for b in range(BL)]
        hT16 = pp.tile([128, BL * 1024], F16, tag="hT16", name="hT16")
        hT8 = pp.tile([128, BL * 1024], F8, tag="hT8", name="hT8")
        hh = pp.tile([33, BL * N], F16, tag="hh", name="hh")
        h = [hh[:, b * N:(b + 1) * N] for b in range(BL)]
        CT = [[pp.tile([128, C], F16, tag=f"CT{b}_{t}", name=f"CT{b}_{t}")
               for t in range(5)] for b in range(BL)]

        b16 = pp.tile([128, 385], F16, tag="b16", name="b16")
        b32 = pp.tile([128, 65], F32, tag="b32", name="b32")
        i128_t = b16[:, 0:128]
        cwtb_t = [b16[0:33, 128 + 32 * l:160 + 32 * l] for l in range(4)]
        fc1w_t = b16[0:C, 256:384]
        fc2w_t = b16[:, 384:385]
        is32_t = b32[0:C, 0:32]
        js32_t = b32[0:C, 32:64]
        fc1b_t = b32[:, 64:65]

        # ------------- DMA schedule (order = queue order) -------------
        nc.sync.dma_start(hT8[:], ht0_d[:])
        for k in range(4):
            nc.sync.dma_start(vt[0][:, VTW // 4 * k:VTW // 4 * (k + 1)],
                              vt_d[0, :, VTW // 4 * k:VTW // 4 * (k + 1)])
        nc.sync.dma_start(vt[1][:, 0:VTW // 2], vt_d[1, :, 0:VTW // 2])
        nc.sync.dma_start(vt[1][:, VTW // 2:], vt_d[1, :, VTW // 2:])

        with tc.tile_pool(name="work", bufs=1) as wk, \
             tc.tile_pool(name="wkps", bufs=1, space="PSUM") as wkps:

            def slab_dma(l):
                sE = wk.tile([64, 9216], F16, tag="sE", bufs=2, name=f"sE{l}")
                sO = wk.tile([64, 9216], F16, tag="sO", bufs=2, name=f"sO{l}")
                nc.sync.dma_start(sE[:], mmw_d[l, 0])
                nc.sync.dma_start(sO[:], mmw_d[l, 1])
                return sE, sO

            slabs_next = slab_dma(0)
            nc.sync.dma_start(b16[:], b16_d[:])
            nc.sync.dma_start(b32[:], b32_d[:])
            nc.sync.dma_start(hh[:], h0c_d[:])
            for b in range(BL):
                vsrc = vi_d[b].rearrange("p (t n) -> p t n", t=5)
                vdst = vinv[b][:].rearrange("p (t n) -> p t n", t=5)
                nc.sync.dma_start(vdst[:, :, 0:N // 2], vsrc[:, :, 0:N // 2])
                nc.sync.dma_start(vdst[:, :, N // 2:], vsrc[:, :, N // 2:])

            pending = []        # deferred transpose emitters from layer l-1
            for l in range(4):
                last = (l == 3)
                sE, sO = slabs_next
                if not last:
                    slabs_next = slab_dma(l + 1)

                # ---- forward NUDFT: fp8 DoubleRow, batch-outer ----
                big = wkps.tile([128, 2048], F32, tag="pxpm", name=f"pxpm{l}")
                for b in range(BL):
                    for q in range(NQ):
                        lhs = hT8[:, 1024 * b + 64 * q:1024 * b + 64 * (q + 1)] \
                            .rearrange("p (two f) -> p two f", two=2)
                        for half in range(2):
                            base = 1216 * q + 608 * half
                            rhs = vt[b][:, base:base + 608].rearrange(
                                "p (two f) -> p two f", two=2)
                            out = big[0:32, 1024 * b + 512 * half:
                                      1024 * b + 512 * half + NWP]
                            nc.tensor.matmul(out, lhs, rhs,
                                             start=(q == 0), stop=(q == NQ - 1),
                                             perf_mode=PM.DoubleRow)

                # ---- x_ft slab ----
                xs2 = wk.tile([64, 2 * NWP], F16, tag="xs2", name=f"xs2_{l}")
                for b in range(BL):
                    nc.vector.tensor_copy(
                        _cap(xs2, 0, 32, [[2, NWP]], b),
                        big[0:32, 1024 * b:1024 * b + NWP])
                    nc.scalar.activation(
                        _cap(xs2, 32, 32, [[2, NWP]], b),
                        big[0:32, 1024 * b + 512:1024 * b + 512 + NWP], AF.Copy)
                # fill the xs2/mix wait with last layer's deferred transposes
                for fn_ in pending[0:4]:
                    fn_()

                # ---- mode mix ----
                pm = big
                for c4 in range(4):
                    for rr in range(36):
                        r = 36 * c4 + rr
                        for par in range(2):
                            u = 2 * r + par
                            mc = mcols[u]
                            st = (sE if par == 0 else sO)
                            nc.tensor.matmul(pm[0:64, 2 * u:2 * u + 2],
                                             st[:, 2304 * c4 + 64 * rr:
                                                2304 * c4 + 64 * (rr + 1)],
                                             xs2[:, 2 * mc:2 * mc + 2],
                                             start=True, stop=True)
                    if c4 == 1 and len(pending) == 8:
                        pending[4](); pending[5]()
                    if c4 == 3 and len(pending) == 8:
                        pending[6](); pending[7]()
                pending = []

                # ---- per batch: extraction -> CT -> inverse; transposes
                #      deferred so PE never waits on a per-group gelu ----
                frs = [wk.tile([C, NWP], F32, tag=f"frs{b}", name=f"frs{l}_{b}")
                       for b in range(BL)]
                fis = [wk.tile([C, NWP], F32, tag=f"fis{b}", name=f"fis{l}_{b}")
                       for b in range(BL)]
                frx = [wk.tile([C, NWP], F32, tag=f"frx{b}", name=f"frx{l}_{b}")
                       for b in range(BL)]
                fix = [wk.tile([C, NWP], F32, tag=f"fix{b}", name=f"fix{l}_{b}")
                       for b in range(BL)]
                tspec = [[(0, 0, 128, 0)], [(0, 128, 128, 0)],
                         [(0, 256, 48, 0), (1, 0, 64, 64)],
                         [(1, 64, 128, 0)], [(1, 192, 112, 0)]]
                pht = wkps.tile([32, 2048], F16, tag="ph", name=f"ph{l}")
                piT2 = wkps.tile([128, 512], F32, tag="piT", name=f"piT{l}")
                ct_eng = 0
                cp_eng = 0
                pc_slot = 0
                ph_slot = 0
                piT_slot = 0

                def extraction(b):
                    nonlocal ct_eng, pc_slot
                    nc.gpsimd.memset(frs[b][:, 288:NWP], 0.0)
                    nc.gpsimd.memset(fis[b][:, 288:NWP], 0.0)
                    nc.gpsimd.memset(frx[b][:], 0.0)
                    nc.gpsimd.memset(fix[b][:], 0.0)
                    nc.vector.tensor_copy(frs[b][:, 0:288],
                                          _cap(pm, 0, 32, [[2, 288]], b))
                    nc.scalar.activation(fis[b][:, 0:288],
                                         _cap(pm, 32, 32, [[2, 288]], b),
                                         AF.Copy)
                    def _cpy(o, i, eng):
                        if eng == 'act':
                            nc.scalar.activation(o, i, AF.Copy)
                        elif eng == 'pool':
                            nc.gpsimd.tensor_copy(o, i)
                        else:
                            nc.vector.tensor_copy(o, i)
                    for (dst, src_, e1, e2) in (
                            (frx[b], frs[b], 'act', 'pool'),
                            (fix[b], fis[b], 'dve', 'pool')):
                        d3 = dst[:, 0:288].rearrange("p (j i) -> p j i", i=24)
                        s3 = src_[:, 0:288].rearrange("p (j i) -> p j i", i=24)
                        _cpy(d3[:, 1:12, 1:12], s3[:, 1:12, 0:11], e1)
                        _cpy(d3[:, 1:12, 13:24], s3[:, 1:12, 12:23], e2)
                        _cpy(d3[:, 1:12, 0:1], s3[:, 1:12, 23:24], e1)
                        _cpy(dst[:, 288:299],
                             s3[:, 11:0:-1, 11:12].rearrange("p j i -> p (j i)"),
                             e1)
                    nc.gpsimd.tensor_scalar(fix[b][:, 288:299],
                                            fix[b][:, 288:299],
                                            -1.0, None, op0=ALU.mult)
                    if l == 0:
                        nc.gpsimd.memset(CT[b][2][32:64, :], 0.0)
                        nc.gpsimd.memset(CT[b][4][96:128, :], 0.0)
                    for t in range(5):
                        for (kind, c0, wdt, r0) in tspec[t]:
                            sd = frs[b] if kind == 0 else fis[b]
                            sf = frx[b] if kind == 0 else fix[b]
                            pc = big[:, 1024 + 32 * pc_slot:1056 + 32 * pc_slot]
                            pc_slot = (pc_slot + 1) % 4
                            nc.tensor.matmul(pc[0:wdt, :], sd[:, c0:c0 + wdt],
                                             is32_t, start=True, stop=False,
                                             is_transpose=True)
                            nc.tensor.matmul(pc[0:wdt, :], sf[:, c0:c0 + wdt],
                                             js32_t, start=False, stop=True,
                                             is_transpose=True)
                            dstap = CT[b][t][r0:r0 + wdt, :]
                            if ct_eng == 1:
                                nc.scalar.activation(dstap, pc[0:wdt, :],
                                                     AF.Copy,
                                                     scale=1.0 / 2048.0)
                            else:
                                nc.vector.tensor_scalar(dstap, pc[0:wdt, :],
                                                        1.0 / 2048.0, None,
                                                        op0=ALU.mult)
                            ct_eng = (ct_eng + 1) % 2

                def inverse(b, pys=None):
                    nonlocal piT_slot
                    for g4 in range(8):
                        piT4 = piT2[:, 128 * piT_slot:128 * (piT_slot + 1)]
                        piT_slot = (piT_slot + 1) % 4
                        for j in range(4):
                            ch = 4 * g4 + j
                            for t in range(5):
                                nc.tensor.matmul(
                                    piT4[:, 32 * j:32 * (j + 1)],
                                    vinv[b][:, N * t + 128 * ch:
                                            N * t + 128 * (ch + 1)],
                                    CT[b][t][:],
                                    start=(t == 0), stop=False)
                            nc.tensor.matmul(
                                piT4[:, 32 * j:32 * (j + 1)],
                                h[b][:, 128 * ch:128 * (ch + 1)],
                                cwtb_t[l], start=False, stop=True)
                        if g4 % 2 == 0:
                            continue
                        # one act/cast per pair of groups (slots adjacent)
                        s0 = (piT_slot - 2) % 4
                        src2 = piT2[:, 128 * s0:128 * s0 + 256]
                        dst2 = hT16[:, 1024 * b + 128 * (g4 - 1):
                                    1024 * b + 128 * (g4 + 1)]
                        if last:
                            nc.vector.tensor_copy(dst2, src2)
                            if pys is not None:
                                transpose_group(b, g4 - 1)
                                transpose_group(b, g4)
                                head_chunk(b, g4 // 2, pys)
                        else:
                            nc.scalar.activation(dst2, src2, AF.Gelu)
                            nc.gpsimd.tensor_copy(
                                hT8[:, 1024 * b + 128 * (g4 - 1):
                                    1024 * b + 128 * (g4 + 1)], dst2)

                def head_chunk(b, c4, pys):
                    pg = big[:, 1024 * (c4 % 2):1024 * (c4 % 2) + 1024]
                    for hhh in range(2):
                        nc.tensor.matmul(pg[:, 512 * hhh:512 * (hhh + 1)],
                                         fc1w_t,
                                         h[b][0:32, 1024 * c4 + 512 * hhh:
                                              1024 * c4 + 512 * (hhh + 1)],
                                         start=True, stop=True)
                    g = wk.tile([128, 1024], F16, tag="g", bufs=2,
                                name=f"g{b}_{c4}")
                    nc.scalar.activation(g[:], pg[:], AF.Gelu, bias=fc1b_t)
                    for k in range(8):
                        nc.tensor.matmul(
                            pys[:, 32 * b + 8 * c4 + k:32 * b + 8 * c4 + k + 1],
                            g[:, 128 * k:128 * (k + 1)],
                            fc2w_t, start=True, stop=True)

                def transpose_group(b, g4):
                    nonlocal cp_eng, ph_slot
                    ph = pht[0:32, 512 * ph_slot:512 * (ph_slot + 1)]
                    ph_slot = (ph_slot + 1) % 4
                    for j in range(4):
                        ch = 4 * g4 + j
                        nc.tensor.matmul(
                            ph[:, 128 * j:128 * (j + 1)],
                            hT16[:, 1024 * b + 32 * ch:
                                 1024 * b + 32 * (ch + 1)],
                            i128_t, start=True, stop=True,
                            is_transpose=True)
                    dst = h[b][0:32, 512 * g4:512 * (g4 + 1)]
                    if last or cp_eng == 0:
                        nc.vector.tensor_copy(dst, ph[:])
                    else:
                        nc.scalar.activation(dst, ph[:], AF.Copy)
                    cp_eng = (cp_eng + 1) % 2

                def transposes(b, pys=None):
                    for g4 in range(8):
                        transpose_group(b, g4)
                        if pys is not None and g4 % 2 == 1:
                            head_chunk(b, g4 // 2, pys)

                extraction(0)
                if last:
                    extraction(1)
                    pys = wkps.tile([128, 64], F32, tag="pys", name="pys")
                    inverse(0, pys)
                    ys0 = wk.tile([128, 32], F32, tag="ys0", name="ys0")
                    nc.vector.tensor_copy(ys0[:], pys[:, 0:32])
                    nc.sync.dma_start(y_d[0], ys0[:])
                    inverse(1, pys)
                    ys1 = wk.tile([128, 32], F32, tag="ys1", name="ys1")
                    nc.vector.tensor_copy(ys1[:], pys[:, 32:64])
                    nc.sync.dma_start(y_d[1], ys1[:])
                else:
                    inverse(0)
                    extraction(1)
                    transposes(0)
                    inverse(1)
                    # b1's hT16->h transposes are only needed by the NEXT
                    # layer's conv; defer them into its fwd/xs2/mix windows.
                    pending = [lambda g4=g4: transpose_group(1, g4)
                               for g4 in range(8)]

    nc.compile()
    return nc


# --------------------------------------------------------------------------
# host marshaling
# --------------------------------------------------------------------------
def _marshal(pos, fc0_w, fc0_b, sw1r, sw1i, sw2r, sw2i, cw, cb,
             fc1_w, fc1_b, fc2_w, fc2_b):
    xp = (pos[:, :, 0] - pos[:, :, 0].min()).astype(np.float64)
    yp = (pos[:, :, 1] - pos[:, :, 1].min()).astype(np.float64)
    sx = np.float64(np.float32(6.28) / np.float32(xp.max()))
    sy = np.float64(np.float32(6.28) / np.float32(yp.max()))
    kx = np.concatenate([np.arange(MODES), np.arange(-MODES, 0)]).astype(np.float64)
    ky = np.concatenate([np.arange(MODES), np.arange(-(MODES - 1), 0)]).astype(np.float64)

    def wrap(v):
        return v - 2 * np.pi * np.round(v / (2 * np.pi))

    axw = np.stack([wrap(kx[i] * sx * xp).astype(np.float32) for i in range(24)],
                   axis=1)
    ayw = np.stack([wrap(ky[j] * sy * yp).astype(np.float32) for j in range(23)],
                   axis=1)

    worder = _w_rows()
    iw = np.array([m % 24 for m in worder])
    jw = np.array([m // 24 for m in worder])
    ph = axw[:, iw, :].astype(np.float64) + ayw[:, jw, :]
    cosW = np.zeros((B, NWP, N), np.float32)
    sinW = np.zeros((B, NWP, N), np.float32)
    cosW[:, :NW] = np.cos(ph)
    sinW[:, :NW] = -np.sin(ph)

    cs = np.stack([cosW, sinW], axis=1)                     # [B, half, NWP, N]
    csb = cs.reshape(B, 2, NWP, NQ, 2, 128)
    vt8 = np.ascontiguousarray(
        csb.transpose(0, 5, 3, 1, 4, 2)
    ).reshape(B, 128, VTW).astype(F8NP)

    vpk = np.zeros((B, 640, N), np.float32)
    vpk[:, 0:NWP] = cosW
    vpk[:, 320:320 + NWP] = sinW
    vi8 = np.ascontiguousarray(
        vpk.reshape(B, 5, 128, N).transpose(0, 2, 1, 3)
    ).reshape(B, 128, 5 * N).astype(F8NP)

    xin = np.stack([xp, yp], axis=-1)
    h0 = (xin @ fc0_w.astype(np.float64) + fc0_b.astype(np.float64))
    ht0 = np.ascontiguousarray(
        h0.reshape(B, 32, 128, C).transpose(0, 2, 1, 3)
    ).reshape(B, 128, 1024).astype(F8NP)
    h0c = np.zeros((B, 33, N), np.float16)
    h0c[:, 0:C] = h0.transpose(0, 2, 1).astype(np.float16)
    h0c[:, 32] = 1.0

    mmw2 = np.zeros((4, 2, 64, 9216), np.float16)
    for l in range(4):
        w1 = sw1r[l].astype(np.float64) + 1j * sw1i[l].astype(np.float64)
        w2 = sw2r[l].astype(np.float64) + 1j * sw2i[l].astype(np.float64)
        for u in range(288):
            a, s = u // 12, u % 12
            wm = w1[:, :, a, s] if a < 12 else w2[:, :, a - 12, s]
            wr = wm.real.astype(np.float16)
            wi = wm.imag.astype(np.float16)
            _, cj = mode_col(u)
            r, par = u // 2, u % 2
            blk = mmw2[l, par, :, 64 * r:64 * (r + 1)]
            blk[0:32, 0:32] = wr
            blk[0:32, 32:64] = wi
            if cj:
                blk[32:64, 0:32] = wi
                blk[32:64, 32:64] = -wr
            else:
                blk[32:64, 0:32] = -wi
                blk[32:64, 32:64] = wr

    # packed small-weight blobs
    b16 = np.zeros((128, 385), np.float16)
    b16[:, 0:128] = np.eye(128, dtype=np.float16)
    for l in range(4):
        b16[0:C, 128 + 32 * l:160 + 32 * l] = cw[l].T.astype(np.float16)
        b16[32, 128 + 32 * l:160 + 32 * l] = cb[l].astype(np.float16)
    b16[0:C, 256:384] = fc1_w.astype(np.float16)
    b16[:, 384] = fc2_w.reshape(128).astype(np.float16)
    b32 = np.zeros((128, 65), np.float32)
    eye32 = np.eye(C, dtype=np.float32)
    b32[0:C, 0:32] = eye32
    b32[0:C, 32:64] = eye32[::-1]
    b32[:, 64] = fc1_b.astype(np.float32)

    shared = dict(mmw2=mmw2, b16=b16, b32=b32)
    per_b = dict(vt8=vt8, vi8=vi8, ht0=ht0, h0c=h0c)
    return per_b, shared


def kernel(**inputs):
    per_b, shared = _marshal(**{k: np.asarray(v) for k, v in inputs.items()})

    if 'nc' not in _CACHE:
        _CACHE['nc'] = _build_program()
    nc = _CACHE['nc']

    in_maps = []
    for core in range(NCORES):
        m = dict(shared)
        s = slice(BL * core, BL * (core + 1))
        m['vt8'] = per_b['vt8'][s]
        m['vi8'] = per_b['vi8'][s]
        # ht0: [128, BL*1024] with batch at col offset 1024b
        m['ht0'] = np.ascontiguousarray(
            per_b['ht0'][s].transpose(1, 0, 2).reshape(128, BL * 1024))
        # h0c: [33, BL*N] with batch at col offset N*b
        m['h0c'] = np.ascontiguousarray(
            per_b['h0c'][s].transpose(1, 0, 2).reshape(33, BL * N))
        in_maps.append(m)

    res = run_bass_kernel_spmd(nc, in_maps, list(range(NCORES)), trace=TRACE)
    _CACHE['last_results'] = res

    fc2_b = np.asarray(inputs['fc2_b']).astype(np.float32)
    out = np.zeros((B, N, 1), np.float32)
    for core in range(NCORES):
        yv = res.results[core]['y']          # [BL, 128, 32]; n = 128j + p
        out[BL * core:BL * (core + 1), :, 0] = \
            yv.transpose(0, 2, 1).reshape(BL, N)
    out += fc2_b.reshape(1, 1, 1)
    return out
